# revision 18
# baseline (speedup 1.0000x reference)
"""Trainium2 Bass kernel for BinaryLinear: out = x @ sign(W).T

Shapes (hardcoded): x [32768, 2048] f32, weight [2048, 2048] f32,
out [32768, 2048] f32.

Strategy: data-parallel over 8 NeuronCores — shard the 32768-token
axis (4096 tokens/core) and replicate the weight. The sharding step on
the host also picks the device-friendly layouts (pure data movement —
all arithmetic stays on device):
  - x is fed per-core as xt[tt, i_p, ic, t_l] = x[128*tt + t_l,
    128*ic + i_p]: each token tile loads as four quarter-K strided
    DMAs (2 KB/partition lines) whose SBUF image is directly the
    pre-transposed stationary operand the PE wants (the systolic
    array contracts over the partition axis); GpSimd casts each
    quarter f32 -> bf16 off the DVE's critical path.
  - weight is fed as W.T [in, out] so sign(W).T is produced on-chip by
    a single ScalarE Sign activation pass (f32 -> bf16) per 128-row
    tile, no transposes.

Per core: 2048 bf16 matmuls ([128,128] stationary x-chunk, [128,512]
moving sign-weight, f32 PSUM) is the PE roofline (~220 ns each warm =
2.33 GHz x 512 moving rows; fp8 DoubleRow measures the *same* MAC rate
on this HW, so bf16 is optimal). The schedule exists to keep the PE at
that cadence end to end:
  - The 16 MB weight load is the long pole at startup (~50+ us at HBM
    rate, slower still while the DMA engines ramp). The first CH=8
    token tiles run in three partial-K passes (ics 0-2 / 3-7 / 8-15)
    staged to the W stream: each pass sweeps all CH tiles, so every
    landed sw tile is re-consumed once per tile-pair PSUM rotation
    instead of once (PSUM's 8 banks cap one rotation at 8 matmuls,
    which would starve at the HBM rate). Partials park in SBUF as
    bf16 and merge on later drains (DVE copy / RMW-add /
    add-on-drain). Remaining tiles run plain 16-ic accumulation.
  - Queues: Sync carries x tiles (+ odd W tiles early), Scalar carries
    even W tiles then the output stores, so stores never delay x.
  - All 8 PSUM banks rotate (pool bufs=8) so a tile's first matmul
    never waits on the previous tile's drain.
  - The last tiles drain+store per 512-column chunk so the final
    store is small and starts as soon as its output chunk stops.
DMA moves 83 MB/core, well under the ~360 GB/s core bandwidth.
"""

import sys

if "/opt/trn_rl_repo" not in sys.path:
    sys.path.insert(0, "/opt/trn_rl_repo")

import numpy as np

T, I, O = 32768, 2048, 2048
NCORES = 8
TL = T // NCORES  # tokens per core

_NC = None


def _build():
    import concourse.bacc as bacc
    import concourse.mybir as mybir
    from concourse import tile
    from contextlib import ExitStack

    f32 = mybir.dt.float32
    bf16 = mybir.dt.bfloat16

    IC = I // 128  # i-chunks (contraction)
    NT = TL // 128  # token tiles per core
    OCW = 512  # matmul moving free dim
    NOC = O // OCW
    CH = 8  # tiles processed as two half-K passes during the W chase
    ICH = IC // 2

    nc = bacc.Bacc("TRN2", target_bir_lowering=False, debug=False, num_devices=NCORES)
    xt = nc.dram_tensor("xt", [NT, 128, IC, 128], f32, kind="ExternalInput")
    wt = nc.dram_tensor("wt", [I, O], f32, kind="ExternalInput")
    out = nc.dram_tensor("out", [TL, O], f32, kind="ExternalOutput")

    with tile.TileContext(nc) as tc, ExitStack() as ctx:
        # sign(W).T resident in SBUF as IC tiles of [128 i, O] bf16
        swt_pool = ctx.enter_context(tc.tile_pool(name="swt", bufs=1))
        swT = [swt_pool.tile([128, O], bf16, name=f"swT{ic}") for ic in range(IC)]

        wprep = ctx.enter_context(tc.tile_pool(name="wprep", bufs=1))
        w_f32 = [
            wprep.tile([128, O], f32, tag="w_f32", name=f"w_f32_{ic}", bufs=3)
            for ic in range(IC)
        ]

        xpool = ctx.enter_context(tc.tile_pool(name="xpool", bufs=3))
        opool = ctx.enter_context(tc.tile_pool(name="opool", bufs=2))
        ppool = ctx.enter_context(tc.tile_pool(name="ppool", bufs=1))
        psum_mm = ctx.enter_context(tc.tile_pool(name="psum_mm", bufs=8, space="PSUM"))

        xTs = [None] * NT
        psb = [ppool.tile([128, O], bf16, name=f"psb{t}") for t in range(CH)]

        def alloc_xT(tt):
            xTs[tt] = xpool.tile(
                [128, IC, 128], bf16, tag="xT", name=f"xT_{tt}", bufs=11
            )

        def load_cast_xq(tt, q):
            # quarter-granular x load: 2 KB/partition strided DMA, cast on
            # GpSimd. Chase passes consume quarters as they land; keeps the
            # early HBM window free for the 16 MB weight stream.
            if xTs[tt] is None:
                alloc_xT(tt)
            xq = xpool.tile([128, 4, 128], f32, tag="xq", name=f"xq_{tt}_{q}", bufs=6)
            nc.sync.dma_start(xq[:], xt[tt][:, 4 * q : 4 * (q + 1), :])
            eng = nc.vector if (tt < CH and q == 0) else nc.gpsimd
            eng.tensor_copy(xTs[tt][:, 4 * q : 4 * (q + 1), :], xq[:])

        def load_cast_x(tt):
            for q in range(4):
                load_cast_xq(tt, q)

        def load_w(ic):
            eng = nc.scalar if ic % 2 == 0 else nc.sync
            eng.dma_start(w_f32[ic][:], wt[128 * ic : 128 * (ic + 1), :])

        def sign_w(ic):
            nc.scalar.activation(
                swT[ic][:], w_f32[ic][:], mybir.ActivationFunctionType.Sign
            )

        # Queue choreography. Chase phase A1 (ics 0-3, all CH tiles) needs
        # only the q0 x-quarters plus sw0-3; A2 needs q1 + sw4-7; pass C
        # needs q2/q3 + sw8-15. Sync carries x quarters + odd W tiles at
        # matching depth; Scalar carries even W (w0 split in half so the
        # first sign lands earliest) then signs, placed so their semaphore
        # waits never starve W descriptor issue.
        nc.scalar.dma_start(w_f32[0][:, 0:1024], wt[0:128, 0:1024])
        nc.scalar.dma_start(w_f32[0][:, 1024:2048], wt[0:128, 1024:2048])
        Sign = mybir.ActivationFunctionType.Sign
        order = [
            ("xq", 0, 0), ("xq", 1, 0),
            ("s0a",), ("s0b",),
            ("w", 2), ("w", 1),
            ("xq", 2, 0), ("xq", 3, 0),
            ("w", 4), ("s", 1), ("s", 2),
            ("w", 3), ("xq", 4, 0), ("xq", 5, 0), ("xq", 6, 0), ("xq", 7, 0),
            ("w", 6), ("s", 3), ("s", 4),
            ("w", 5), ("xq", 0, 1), ("xq", 1, 1),
            ("w", 8), ("s", 5), ("s", 6),
            ("w", 7), ("xq", 2, 1), ("xq", 3, 1),
            ("w", 10), ("s", 7),
            ("w", 9), ("xq", 4, 1), ("xq", 5, 1), ("xq", 6, 1), ("xq", 7, 1),
            ("w", 12), ("s", 8), ("s", 9),
            ("w", 11), ("xq", 0, 2), ("xq", 1, 2), ("xq", 2, 2), ("xq", 3, 2),
            ("w", 14), ("s", 10), ("s", 11),
            ("w", 13), ("xq", 4, 2), ("xq", 5, 2), ("xq", 6, 2), ("xq", 7, 2),
            ("s", 12), ("s", 13),
            ("xq", 0, 3), ("xq", 1, 3), ("xq", 2, 3), ("xq", 3, 3),
            ("w", 15),
            ("xq", 4, 3), ("xq", 5, 3), ("xq", 6, 3), ("xq", 7, 3),
            ("s", 14), ("s", 15),
        ]
        for item in order:
            if item[0] == "w":
                load_w(item[1])
            elif item[0] == "xq":
                load_cast_xq(item[1], item[2])
            elif item[0] == "s":
                sign_w(item[1])
            elif item[0] == "s0a":
                nc.scalar.activation(swT[0][:, 0:1024], w_f32[0][:, 0:1024], Sign)
            elif item[0] == "s0b":
                nc.scalar.activation(swT[0][:, 1024:2048], w_f32[0][:, 1024:2048], Sign)

        def mm_block(accs, tt, ic_lo, ic_hi):
            for ic in range(ic_lo, ic_hi):
                for oc in range(NOC):
                    nc.tensor.matmul(
                        accs[oc][:],
                        xTs[tt][:, ic, :],
                        swT[ic][:, OCW * oc : OCW * (oc + 1)],
                        start=(ic == ic_lo),
                        stop=(ic == ic_hi - 1),
                    )

        def new_accs(tt, tag):
            return [
                psum_mm.tile([128, OCW], f32, tag="acc", name=f"acc_{tag}_{tt}_{oc}")
                for oc in range(NOC)
            ]

        def store_tile(tt, accs, add_psb=None, per_oc=False, q=None):
            o_sb = opool.tile([128, O], f32, tag="o_sb", name=f"o_sb_{tt}")
            for oc in range(NOC):
                dst = o_sb[:, OCW * oc : OCW * (oc + 1)]
                if add_psb is not None:
                    nc.vector.scalar_tensor_tensor(
                        dst,
                        accs[oc][:],
                        1.0,
                        add_psb[:, OCW * oc : OCW * (oc + 1)],
                        mybir.AluOpType.mult,
                        mybir.AluOpType.add,
                    )
                else:
                    nc.vector.tensor_copy(dst, accs[oc][:])
                if per_oc:
                    (q or nc.scalar).dma_start(
                        out[128 * tt : 128 * (tt + 1), OCW * oc : OCW * (oc + 1)],
                        dst,
                    )
            if not per_oc:
                (q or nc.scalar).dma_start(out[128 * tt : 128 * (tt + 1), :], o_sb[:])

        # ---- W-chase: the first CH tiles run in three partial-K passes
        # so the PE re-consumes already-landed sw tiles across many PSUM
        # group rotations instead of starving at 8 matmuls per sw tile.
        # Pass A: ics 0-3 for all CH tiles -> psb (bf16 partial).
        # Pass B: ics 4-7 for all CH tiles -> psb += acc (DVE RMW).
        # Pass C: ics 8-15, final add happens in the drain.
        groups = [(2 * p, 2 * p + 1) for p in range(CH // 2)]
        if CH % 2:
            groups.append((CH - 1,))
        for phase, (lo, hi) in enumerate([(0, 3), (3, 8)]):
            for grp in groups:
                pa = [new_accs(tt, f"a{phase}") for tt in grp]
                for ic in range(lo, hi):
                    for k, tt in enumerate(grp):
                        for oc in range(NOC):
                            nc.tensor.matmul(
                                pa[k][oc][:],
                                xTs[tt][:, ic, :],
                                swT[ic][:, OCW * oc : OCW * (oc + 1)],
                                start=(ic == lo),
                                stop=(ic == hi - 1),
                            )
                for k, tt in enumerate(grp):
                    for oc in range(NOC):
                        sl = psb[tt][:, OCW * oc : OCW * (oc + 1)]
                        if phase == 0:
                            nc.vector.tensor_copy(sl, pa[k][oc][:])
                        else:
                            nc.vector.scalar_tensor_tensor(
                                sl,
                                pa[k][oc][:],
                                1.0,
                                sl,
                                mybir.AluOpType.mult,
                                mybir.AluOpType.add,
                            )

        # ---- chase final pass: ics 8..15, pair-major like pass A so a
        # late sw tile stalls a pair's 14 us sweep, not one tile's 7 us;
        # the drain adds psb ----
        for grp in groups:
            pb = [new_accs(tt, "b") for tt in grp]
            for ic in range(ICH, IC):
                for k, tt in enumerate(grp):
                    for oc in range(NOC):
                        nc.tensor.matmul(
                            pb[k][oc][:],
                            xTs[tt][:, ic, :],
                            swT[ic][:, OCW * oc : OCW * (oc + 1)],
                            start=(ic == ICH),
                            stop=(ic == IC - 1),
                        )
            for k, tt in enumerate(grp):
                store_tile(tt, pb[k], add_psb=psb[tt])

        # ---- steady state: full 16-ic accumulation ----
        for tt in range(CH, NT):
            load_cast_x(tt)
            accs = new_accs(tt, "s")
            for oc in range(NOC):
                for ic in range(IC):
                    nc.tensor.matmul(
                        accs[oc][:],
                        xTs[tt][:, ic, :],
                        swT[ic][:, OCW * oc : OCW * (oc + 1)],
                        start=(ic == 0),
                        stop=(ic == IC - 1),
                    )
            store_tile(
                tt, accs, per_oc=(tt >= NT - 2), q=nc.sync if tt >= NT - 4 else None
            )

    nc.compile()
    return nc


def _get_nc():
    global _NC
    if _NC is None:
        _NC = _build()
    return _NC


def _in_maps(x, w):
    x = np.asarray(x, dtype=np.float32)
    w = np.asarray(w, dtype=np.float32)
    assert x.shape == (T, I) and w.shape == (O, I)
    # xt[tt, i_p, ic, t_l] = x[128*tt + t_l, 128*ic + i_p]
    xt = np.ascontiguousarray(
        x.reshape(T // 128, 128, I // 128, 128).transpose(0, 3, 2, 1)
    )
    wt = np.ascontiguousarray(w.T)
    ntl = TL // 128  # token tiles per core
    return [
        {"xt": xt[c * ntl : (c + 1) * ntl], "wt": wt} for c in range(NCORES)
    ]


def kernel(**inputs):
    from concourse.bass_utils import run_bass_kernel_spmd

    nc = _get_nc()
    res = run_bass_kernel_spmd(
        nc, _in_maps(inputs["x"], inputs["weight"]), core_ids=list(range(NCORES))
    )
    return np.concatenate([r["out"] for r in res.results], axis=0)
